# revision 3
# baseline (speedup 1.0000x reference)
"""Trainium2 Bass kernel: dot-product attention scores (matvec).

scores = encoder_out[16384, 4096] @ decoder_hidden[-1][4096] -> [16384]

Sharding: encoder_out row-wise across 8 cores (2048 rows each),
decoder_hidden replicated. No cross-core communication.

Per-core kernel (memory-bound, ~32 MB of HBM reads), raw Bass with
manual semaphores (the TileContext tail drain does not compile with
this walrus build, and the fused raw-ISA DVE reduce ops are rejected
by codegen, so the compute is split across two engines):

  sync (SP/HWDGE):  broadcast t[4096] to 128 partitions once, then
                    stream encoder rows as [128, CHUNK*4096] tiles
                    (CHUNK row-blocks per dma_start -> >=4 MB per DMA
                    for peak HBM bandwidth), triple-buffered.
  vector (DVE):     prod = enc_block * t_bcast   (one pass / block)
  scalar (ACT):     activation(Copy, accum_out) -> per-partition sum
                    over the 4096 free elems = the dot products;
                    finally DMAs the [128, 16] score tile out.

Per-core output is [128, 16] with out[p, n] = scores[n*128 + p];
the host transposes/flattens and concatenates the 8 shards.
"""

import numpy as np

S, H, L = 16384, 4096, 2
N_CORES = 8
S_LOC = S // N_CORES        # 2048 rows per core
P = 128                     # SBUF partitions
N_BLOCKS = S_LOC // P       # 16 row-blocks per core
CHUNK = 2                   # row-blocks per DMA (4 MB per dma_start)
N_LOADS = N_BLOCKS // CHUNK
NBUF = 3                    # enc tile buffers (triple buffering)
PBUF = 2                    # product buffers (DVE -> ACT handoff)

_NC_CACHE = {}
LAST_RESULT = None          # BassKernelResults of the most recent run


def _build_nc():
    import concourse.bass as bass
    from concourse import mybir

    f32 = mybir.dt.float32

    nc = bass.Bass(trn_type="TRN2")
    enc = nc.dram_tensor("enc", [S_LOC, H], f32, kind="ExternalInput")
    dec = nc.dram_tensor("dec", [L, H], f32, kind="ExternalInput")
    out = nc.dram_tensor("out", [P, N_BLOCKS], f32, kind="ExternalOutput")

    # enc rows r = (i*CHUNK + j)*P + p  ->  [i, p, j, h]; per-partition the
    # DMA reads CHUNK contiguous 16 KB runs.
    enc_r = enc.rearrange("(i j p) h -> i p j h", j=CHUNK, p=P)

    with (
        nc.sbuf_tensor("tb", [P, H], f32) as tb,
        nc.sbuf_tensor("ebuf0", [P, CHUNK * H], f32) as ebuf0,
        nc.sbuf_tensor("ebuf1", [P, CHUNK * H], f32) as ebuf1,
        nc.sbuf_tensor("ebuf2", [P, CHUNK * H], f32) as ebuf2,
        nc.sbuf_tensor("prod0", [P, H], f32) as prod0,
        nc.sbuf_tensor("prod1", [P, H], f32) as prod1,
        nc.sbuf_tensor("junk", [P, H], mybir.dt.bfloat16) as junk,
        nc.sbuf_tensor("sc", [P, N_BLOCKS], f32) as sc,
        nc.semaphore("tb_sem") as tb_sem,
        nc.semaphore("esem0") as esem0,
        nc.semaphore("esem1") as esem1,
        nc.semaphore("esem2") as esem2,
        nc.semaphore("mul_sem") as mul_sem,
        nc.semaphore("red_sem") as red_sem,
        nc.semaphore("store_sem") as store_sem,
        nc.Block() as block,
    ):
        ebufs = [ebuf0, ebuf1, ebuf2]
        # One DMA sem per enc buffer slot. A single sem counted cumulatively
        # across in-flight transfers is racy: each dma_start's 16 incs come
        # from 16 independent SDMA engines, so sem >= 16*(i+1) can be reached
        # with transfer i still incomplete (a lagging engine) while later
        # transfers contribute incs. Per-slot sems are safe because a slot's
        # next transfer is only issued after the consumer drained the
        # previous one (mul_sem back-pressure below).
        esems = [esem0, esem1, esem2]
        prods = [prod0, prod1]
        assert len(ebufs) == NBUF and len(prods) == PBUF

        @block.sync
        def _(sync):
            sync.dma_start(tb[:], dec[L - 1 : L, :].to_broadcast((P, H))).then_inc(
                tb_sem, 16
            )
            for i in range(N_LOADS):
                if i >= NBUF:
                    # enc slot reuse: all DVE muls of load i-NBUF must be done
                    sync.wait_ge(mul_sem, (i - NBUF + 1) * CHUNK)
                sync.dma_start(ebufs[i % NBUF][:], enc_r[i]).then_inc(
                    esems[i % NBUF], 16
                )
            # final store: gated on ACT having produced all 16 sums
            sync.wait_ge(red_sem, N_BLOCKS)
            sync.dma_start(out[:], sc[:]).then_inc(store_sem, 16)
            sync.wait_ge(store_sem, 16)

        @block.vector
        def _(vector):
            vector.wait_ge(tb_sem, 16)
            for n in range(N_BLOCKS):
                i = n // CHUNK
                j = n % CHUNK
                vector.wait_ge(esems[i % NBUF], 16 * (i // NBUF + 1))
                if n >= PBUF:
                    # prod slot reuse: ACT must have consumed block n-PBUF
                    vector.wait_ge(red_sem, n - PBUF + 1)
                nc.vector.tensor_mul(
                    prods[n % PBUF][:],
                    ebufs[i % NBUF][:, j * H : (j + 1) * H],
                    tb[:],
                ).then_inc(mul_sem, 1)

        @block.scalar
        def _(scalar):
            for n in range(N_BLOCKS):
                scalar.wait_ge(mul_sem, n + 1)
                nc.scalar.activation(
                    out=junk[:],
                    in_=prods[n % PBUF][:],
                    func=mybir.ActivationFunctionType.Copy,
                    accum_out=sc[:, n : n + 1],
                ).then_inc(red_sem, 1)

    return nc


def kernel(encoder_out: np.ndarray, decoder_hidden: np.ndarray) -> np.ndarray:
    global LAST_RESULT
    from concourse.bass_utils import run_bass_kernel_spmd

    encoder_out = np.ascontiguousarray(np.asarray(encoder_out, dtype=np.float32))
    decoder_hidden = np.ascontiguousarray(np.asarray(decoder_hidden, dtype=np.float32))

    if "nc" not in _NC_CACHE:
        _NC_CACHE["nc"] = _build_nc()
    nc = _NC_CACHE["nc"]

    in_maps = [
        {"enc": encoder_out[c * S_LOC : (c + 1) * S_LOC], "dec": decoder_hidden}
        for c in range(N_CORES)
    ]
    res = run_bass_kernel_spmd(nc, in_maps, core_ids=list(range(N_CORES)))
    LAST_RESULT = res

    # out[p, n] = scores[n*128 + p] within each shard
    parts = [np.asarray(r["out"]).T.reshape(-1) for r in res.results]
    return np.concatenate(parts).astype(np.float32)


# revision 10
# speedup vs baseline: 1.0751x; 1.0751x over previous
"""Trainium2 Bass kernel: dot-product attention scores (matvec).

scores = encoder_out[16384, 4096] @ decoder_hidden[-1][4096] -> [16384]

Sharding: encoder_out row-wise across 8 cores (2048 rows each),
decoder_hidden replicated. No cross-core communication.

Per-core kernel (memory-bound, ~32 MB of HBM reads), raw Bass with
manual semaphores (the TileContext tail drain does not compile with
this walrus build, and the fused raw-ISA DVE reduce ops are rejected
by codegen, so the compute is split across two engines):

  sync (SP/HWDGE):  broadcast t[4096] to 128 partitions once, then
                    stream encoder rows as [128, CHUNK*4096] tiles
                    (CHUNK row-blocks per dma_start -> >=4 MB per DMA
                    for peak HBM bandwidth), triple-buffered.
  vector (DVE):     prod = enc_block * t_bcast   (one pass / block)
  scalar (ACT):     activation(Copy, accum_out) -> per-partition sum
                    over the 4096 free elems = the dot products;
                    finally DMAs the [128, 16] score tile out.

Per-core output is [128, 16] with out[p, n] = scores[n*128 + p];
the host transposes/flattens and concatenates the 8 shards.
"""

import numpy as np

S, H, L = 16384, 4096, 2
N_CORES = 8
S_LOC = S // N_CORES        # 2048 rows per core
P = 128                     # SBUF partitions
N_BLOCKS = S_LOC // P       # 16 row-blocks per core
CHUNK = 2                   # row-blocks per DMA (4 MB per dma_start)
N_LOADS = N_BLOCKS // CHUNK
NBUF = 4                    # enc tile buffers (deep issue-ahead)
PBUF = 2                    # product buffers (DVE -> ACT handoff)

_NC_CACHE = {}
LAST_RESULT = None          # BassKernelResults of the most recent run


def _build_nc():
    import concourse.bass as bass
    from concourse import mybir

    f32 = mybir.dt.float32

    nc = bass.Bass(trn_type="TRN2")
    enc = nc.dram_tensor("enc", [S_LOC, H], f32, kind="ExternalInput")
    dec = nc.dram_tensor("dec", [L, H], f32, kind="ExternalInput")
    out = nc.dram_tensor("out", [P, N_BLOCKS], f32, kind="ExternalOutput")

    # enc rows r = (i*CHUNK + j)*P + p  ->  [i, p, j, h]; per-partition the
    # DMA reads CHUNK contiguous 16 KB runs.
    enc_r = enc.rearrange("(i j p) h -> i p j h", j=CHUNK, p=P)

    with (
        nc.sbuf_tensor("tb", [P, H], f32) as tb,
        nc.sbuf_tensor("ebuf0", [P, CHUNK * H], f32) as ebuf0,
        nc.sbuf_tensor("ebuf1", [P, CHUNK * H], f32) as ebuf1,
        nc.sbuf_tensor("ebuf2", [P, CHUNK * H], f32) as ebuf2,
        nc.sbuf_tensor("ebuf3", [P, CHUNK * H], f32) as ebuf3,
        nc.sbuf_tensor("prod0", [P, H], f32) as prod0,
        nc.sbuf_tensor("prod1", [P, H], f32) as prod1,
        nc.sbuf_tensor("junk", [P, H], mybir.dt.bfloat16) as junk,
        nc.sbuf_tensor("sc", [P, N_BLOCKS], f32) as sc,
        nc.semaphore("tb_sem") as tb_sem,
        nc.semaphore("esem0") as esem0,
        nc.semaphore("esem1") as esem1,
        nc.semaphore("esem2") as esem2,
        nc.semaphore("esem3") as esem3,
        nc.semaphore("mul_sem") as mul_sem,
        nc.semaphore("red_sem") as red_sem,
        nc.semaphore("store_sem") as store_sem,
        nc.Block() as block,
    ):
        ebufs = [ebuf0, ebuf1, ebuf2, ebuf3]
        # One DMA sem per enc buffer slot. A single sem counted cumulatively
        # across in-flight transfers is racy: each dma_start's 16 incs come
        # from 16 independent SDMA engines, so sem >= 16*(i+1) can be reached
        # with transfer i still incomplete (a lagging engine) while later
        # transfers contribute incs. Per-slot sems are safe because a slot's
        # next transfer is only issued after the consumer drained the
        # previous one (mul_sem back-pressure below).
        esems = [esem0, esem1, esem2, esem3]
        prods = [prod0, prod1]
        assert len(ebufs) == NBUF and len(prods) == PBUF

        @block.sync
        def _(sync):
            for i in range(N_LOADS):
                if i >= NBUF:
                    # enc slot reuse: all DVE muls of load i-NBUF must be done
                    sync.wait_ge(mul_sem, (i - NBUF + 1) * CHUNK)
                sync.dma_start(ebufs[i % NBUF][:], enc_r[i]).then_inc(
                    esems[i % NBUF], 16
                )
            # final store: gated on ACT having produced all 16 sums
            sync.wait_ge(red_sem, N_BLOCKS)
            sync.dma_start(out[:], sc[:]).then_inc(store_sem, 16)
            sync.wait_ge(store_sem, 16)

        @block.vector
        def _(vector):
            vector.wait_ge(tb_sem, 16)
            for n in range(N_BLOCKS):
                i = n // CHUNK
                j = n % CHUNK
                vector.wait_ge(esems[i % NBUF], 16 * (i // NBUF + 1))
                if n >= PBUF:
                    # prod slot reuse: ACT must have consumed block n-PBUF
                    vector.wait_ge(red_sem, n - PBUF + 1)
                nc.vector.tensor_mul(
                    prods[n % PBUF][:],
                    ebufs[i % NBUF][:, j * H : (j + 1) * H],
                    tb[:],
                ).then_inc(mul_sem, 1)

        @block.scalar
        def _(scalar):
            # tb broadcast on ACT's HWDGE ring -> concurrent with the first
            # enc load on sync's ring
            scalar.dma_start(tb[:], dec[L - 1 : L, :].to_broadcast((P, H))).then_inc(
                tb_sem, 16
            )
            for n in range(N_BLOCKS):
                scalar.wait_ge(mul_sem, n + 1)
                nc.scalar.activation(
                    out=junk[:],
                    in_=prods[n % PBUF][:],
                    func=mybir.ActivationFunctionType.Copy,
                    accum_out=sc[:, n : n + 1],
                ).then_inc(red_sem, 1)

    return nc


def kernel(encoder_out: np.ndarray, decoder_hidden: np.ndarray) -> np.ndarray:
    global LAST_RESULT
    from concourse.bass_utils import run_bass_kernel_spmd

    encoder_out = np.ascontiguousarray(np.asarray(encoder_out, dtype=np.float32))
    decoder_hidden = np.ascontiguousarray(np.asarray(decoder_hidden, dtype=np.float32))

    if "nc" not in _NC_CACHE:
        _NC_CACHE["nc"] = _build_nc()
    nc = _NC_CACHE["nc"]

    in_maps = [
        {"enc": encoder_out[c * S_LOC : (c + 1) * S_LOC], "dec": decoder_hidden}
        for c in range(N_CORES)
    ]
    res = run_bass_kernel_spmd(nc, in_maps, core_ids=list(range(N_CORES)))
    LAST_RESULT = res

    # out[p, n] = scores[n*128 + p] within each shard
    parts = [np.asarray(r["out"]).T.reshape(-1) for r in res.results]
    return np.concatenate(parts).astype(np.float32)


# revision 17
# speedup vs baseline: 1.1524x; 1.0719x over previous
"""Trainium2 Bass kernel: dot-product attention scores (matvec).

scores = encoder_out[16384, 4096] @ decoder_hidden[-1][4096] -> [16384]

Sharding: encoder_out row-wise across 8 cores (2048 rows each),
decoder_hidden replicated. No cross-core communication.

Per-core kernel (memory-bound, 32 MB of HBM reads), raw Bass with
manual semaphores (the TileContext tail drain does not compile with
this walrus build, and the fused raw-ISA DVE ops — tensor_tensor_reduce,
affine_mul_reduce, partition_broadcast — are rejected by its codegen,
so the compute is split across standard-BIR ops on two engines):

  tensor (PE):      broadcast target row t[4096] to all 128 partitions
                    once: ones[K=1,M=128].T @ t[K=1,N=512] x 8 -> PSUM
  scalar (ACT):     copies the PSUM broadcast to SBUF, then per block
                    activation(Copy, accum_out) -> per-partition sum
                    over 4096 free elems = the dot products
  sync (SP/HWDGE):  streams encoder rows as [128, 4096] tiles, 2 MB
                    per dma_start, 6 buffers deep; issues final store
  vector (DVE):     prod = enc_block * t_bcast (one pass per block)

Sem protocol notes (both learned from races on HW under profiling):
  - one DMA sem per enc buffer slot; a single cumulatively-counted sem
    is racy because each dma_start's 16 incs come from 16 independent
    SDMA engines, so sem >= 16*(i+1) can be hit while transfer i still
    has a lagging engine
  - cross-engine handoffs (ACT accum -> store DMA) must go through a
    semaphore; issuing the store from the producing engine right after
    the producing op raced on HW

Per-core output is [128, 16] with out[p, n] = scores[n*128 + p];
the host transposes/flattens and concatenates the 8 shards.
"""

import numpy as np

S, H, L = 16384, 4096, 2
N_CORES = 8
S_LOC = S // N_CORES        # 2048 rows per core
P = 128                     # SBUF partitions
N_BLOCKS = S_LOC // P       # 16 row-blocks per core = 16 loads of 2 MB
NBUF = 6                    # enc tile buffers (deep issue-ahead)
PBUF = 3                    # product buffers (DVE -> ACT handoff)
MM_N = 512                  # max matmul free dim (one PSUM bank)

_NC_CACHE = {}
LAST_RESULT = None          # BassKernelResults of the most recent run


def _build_nc():
    import concourse.bass as bass
    from concourse import mybir

    f32 = mybir.dt.float32

    nc = bass.Bass(trn_type="TRN2")
    enc = nc.dram_tensor("enc", [S_LOC, H], f32, kind="ExternalInput")
    dec = nc.dram_tensor("dec", [L, H], f32, kind="ExternalInput")
    out = nc.dram_tensor("out", [P, N_BLOCKS], f32, kind="ExternalOutput")

    # enc row r = n*P + p  ->  [n, p, h]; per-partition one contiguous 16 KB run
    enc_r = enc.rearrange("(n p) h -> n p h", p=P)

    from contextlib import ExitStack

    with ExitStack() as ctx:
        trow = ctx.enter_context(nc.sbuf_tensor("trow", [1, H], f32))
        ones = ctx.enter_context(nc.sbuf_tensor("ones", [1, P], f32))
        tb = ctx.enter_context(nc.sbuf_tensor("tb", [P, H], f32))
        ebufs = [
            ctx.enter_context(nc.sbuf_tensor(f"ebuf{i}", [P, H], f32))
            for i in range(NBUF)
        ]
        prods = [
            ctx.enter_context(nc.sbuf_tensor(f"prod{i}", [P, H], f32))
            for i in range(PBUF)
        ]
        junk = ctx.enter_context(nc.sbuf_tensor("junk", [P, H], mybir.dt.bfloat16))
        sc = ctx.enter_context(nc.sbuf_tensor("sc", [P, N_BLOCKS], f32))
        pb = ctx.enter_context(nc.psum_tensor("pb", [P, H], f32))
        trow_sem = ctx.enter_context(nc.semaphore("trow_sem"))
        ones_sem = ctx.enter_context(nc.semaphore("ones_sem"))
        mm_sem = ctx.enter_context(nc.semaphore("mm_sem"))
        tb_sem = ctx.enter_context(nc.semaphore("tb_sem"))
        esems = [
            ctx.enter_context(nc.semaphore(f"esem{i}")) for i in range(NBUF)
        ]
        mul_sem = ctx.enter_context(nc.semaphore("mul_sem"))
        red_sem = ctx.enter_context(nc.semaphore("red_sem"))
        store_sem = ctx.enter_context(nc.semaphore("store_sem"))
        block = ctx.enter_context(nc.Block())

        @block.sync
        def _(sync):
            for i in range(N_BLOCKS):
                if i >= NBUF:
                    # slot reuse: DVE must have consumed load i-NBUF
                    sync.wait_ge(mul_sem, i - NBUF + 1)
                sync.dma_start(ebufs[i % NBUF][:], enc_r[i]).then_inc(
                    esems[i % NBUF], 16
                )
            sync.wait_ge(red_sem, N_BLOCKS)
            sync.dma_start(out[:], sc[:]).then_inc(store_sem, 16)
            sync.wait_ge(store_sem, 16)

        @block.tensor
        def _(tensor):
            # broadcast t across partitions: out[m, n] = ones[m] * trow[n]
            tensor.wait_ge(trow_sem, 16)
            tensor.wait_ge(ones_sem, 1)
            for k in range(H // MM_N):
                tensor.matmul(
                    pb[:, k * MM_N : (k + 1) * MM_N],
                    ones[0:1, :],
                    trow[0:1, k * MM_N : (k + 1) * MM_N],
                ).then_inc(mm_sem, 1)

        @block.vector
        def _(vector):
            nc.vector.memset(ones[0:1, :], 1.0).then_inc(ones_sem, 1)
            vector.wait_ge(tb_sem, 1)
            for n in range(N_BLOCKS):
                vector.wait_ge(esems[n % NBUF], 16 * (n // NBUF + 1))
                if n >= PBUF:
                    # prod slot reuse: ACT must have consumed block n-PBUF
                    vector.wait_ge(red_sem, n - PBUF + 1)
                nc.vector.tensor_mul(
                    prods[n % PBUF][:], ebufs[n % NBUF][:], tb[:]
                ).then_inc(mul_sem, 1)

        @block.scalar
        def _(scalar):
            # 16 KB target-row load on ACT's HWDGE ring (sync's ring busy
            # with the enc stream)
            scalar.dma_start(trow[0:1, :], dec[L - 1 : L, :]).then_inc(trow_sem, 16)
            scalar.wait_ge(mm_sem, H // MM_N)
            nc.scalar.activation(
                out=tb[:], in_=pb[:], func=mybir.ActivationFunctionType.Copy
            ).then_inc(tb_sem, 1)
            for n in range(N_BLOCKS):
                scalar.wait_ge(mul_sem, n + 1)
                nc.scalar.activation(
                    out=junk[:],
                    in_=prods[n % PBUF][:],
                    func=mybir.ActivationFunctionType.Copy,
                    accum_out=sc[:, n : n + 1],
                ).then_inc(red_sem, 1)

    return nc


def kernel(encoder_out: np.ndarray, decoder_hidden: np.ndarray) -> np.ndarray:
    global LAST_RESULT
    from concourse.bass_utils import run_bass_kernel_spmd

    encoder_out = np.ascontiguousarray(np.asarray(encoder_out, dtype=np.float32))
    decoder_hidden = np.ascontiguousarray(np.asarray(decoder_hidden, dtype=np.float32))

    if "nc" not in _NC_CACHE:
        _NC_CACHE["nc"] = _build_nc()
    nc = _NC_CACHE["nc"]

    in_maps = [
        {"enc": encoder_out[c * S_LOC : (c + 1) * S_LOC], "dec": decoder_hidden}
        for c in range(N_CORES)
    ]
    res = run_bass_kernel_spmd(nc, in_maps, core_ids=list(range(N_CORES)))
    LAST_RESULT = res

    # out[p, n] = scores[n*128 + p] within each shard
    parts = [np.asarray(r["out"]).T.reshape(-1) for r in res.results]
    return np.concatenate(parts).astype(np.float32)


# revision 21
# speedup vs baseline: 1.3041x; 1.1316x over previous
"""Trainium2 Bass kernel: dot-product attention scores (matvec).

scores = encoder_out[16384, 4096] @ decoder_hidden[-1][4096] -> [16384]

Sharding: encoder_out row-wise across 8 cores (2048 rows each),
decoder_hidden replicated. No cross-core communication.

Per-core kernel (memory-bound, 32 MB of HBM reads), raw Bass with
manual semaphores (the TileContext tail drain does not compile with
this walrus build, and the fused raw-ISA DVE ops — tensor_tensor_reduce,
affine_mul_reduce, partition_broadcast — are rejected by its codegen,
so the compute is split across standard-BIR ops on two engines):

  sync (SP/HWDGE):  broadcasts the left half of t[4096] to all 128
                    partitions (step-0 DRAM source AP), then streams
                    encoder rows as [128, 4096] tiles, 2 MB per
                    dma_start, 6 buffers deep; issues the final store
  scalar (ACT):     broadcasts the right half of t concurrently on its
                    own HWDGE ring, then per block runs
                    activation(Copy, accum_out) -> per-partition sum
                    over 4096 free elems = the dot products
  vector (DVE):     prod = enc_block * t_bcast (one pass per block)

Sem protocol notes (both learned from races on HW under profiling):
  - one DMA sem per enc buffer slot; a single cumulatively-counted sem
    is racy because each dma_start's 16 incs come from 16 independent
    SDMA engines, so sem >= 16*(i+1) can be hit while transfer i still
    has a lagging engine
  - cross-engine handoffs (ACT accum -> store DMA) must go through a
    semaphore; issuing the store from the producing engine right after
    the producing op raced on HW

Per-core output is [128, 16] with out[p, n] = scores[n*128 + p];
the host transposes/flattens and concatenates the 8 shards.
"""

import numpy as np

S, H, L = 16384, 4096, 2
N_CORES = 8
S_LOC = S // N_CORES        # 2048 rows per core
P = 128                     # SBUF partitions
N_BLOCKS = S_LOC // P       # 16 row-blocks per core = 16 loads of 2 MB
NBUF = 6                    # enc tile buffers (deep issue-ahead)
PBUF = 3                    # product buffers (DVE -> ACT handoff)
MM_N = 512                  # max matmul free dim (one PSUM bank)

_NC_CACHE = {}
LAST_RESULT = None          # BassKernelResults of the most recent run


def _build_nc():
    import concourse.bass as bass
    from concourse import mybir

    f32 = mybir.dt.float32

    nc = bass.Bass(trn_type="TRN2")
    enc = nc.dram_tensor("enc", [S_LOC, H], f32, kind="ExternalInput")
    dec = nc.dram_tensor("dec", [L, H], f32, kind="ExternalInput")
    out = nc.dram_tensor("out", [P, N_BLOCKS], f32, kind="ExternalOutput")

    # enc row r = n*P + p  ->  [n, p, h]; per-partition one contiguous 16 KB run
    enc_r = enc.rearrange("(n p) h -> n p h", p=P)

    from contextlib import ExitStack

    with ExitStack() as ctx:
        tb = ctx.enter_context(nc.sbuf_tensor("tb", [P, H], f32))
        ebufs = [
            ctx.enter_context(nc.sbuf_tensor(f"ebuf{i}", [P, H], f32))
            for i in range(NBUF)
        ]
        prods = [
            ctx.enter_context(nc.sbuf_tensor(f"prod{i}", [P, H], f32))
            for i in range(PBUF)
        ]
        junk = ctx.enter_context(nc.sbuf_tensor("junk", [P, H], mybir.dt.bfloat16))
        sc = ctx.enter_context(nc.sbuf_tensor("sc", [P, N_BLOCKS], f32))
        tbl_sem = ctx.enter_context(nc.semaphore("tbl_sem"))
        tbr_sem = ctx.enter_context(nc.semaphore("tbr_sem"))
        esems = [
            ctx.enter_context(nc.semaphore(f"esem{i}")) for i in range(NBUF)
        ]
        mul_sem = ctx.enter_context(nc.semaphore("mul_sem"))
        red_sem = ctx.enter_context(nc.semaphore("red_sem"))
        store_sem = ctx.enter_context(nc.semaphore("store_sem"))
        block = ctx.enter_context(nc.Block())

        H2 = H // 2

        @block.sync
        def _(sync):
            # left half of the target broadcast ahead of the enc stream
            sync.dma_start(
                tb[:, :H2], dec[L - 1 : L, :H2].to_broadcast((P, H2))
            ).then_inc(tbl_sem, 16)
            for i in range(N_BLOCKS):
                if i >= NBUF:
                    # slot reuse: DVE must have consumed load i-NBUF
                    sync.wait_ge(mul_sem, i - NBUF + 1)
                sync.dma_start(ebufs[i % NBUF][:], enc_r[i]).then_inc(
                    esems[i % NBUF], 16
                )
            sync.wait_ge(red_sem, N_BLOCKS)
            sync.dma_start(out[:], sc[:]).then_inc(store_sem, 16)
            sync.wait_ge(store_sem, 16)

        @block.vector
        def _(vector):
            vector.wait_ge(tbl_sem, 16)
            vector.wait_ge(tbr_sem, 16)
            for n in range(N_BLOCKS):
                vector.wait_ge(esems[n % NBUF], 16 * (n // NBUF + 1))
                if n >= PBUF:
                    # prod slot reuse: ACT must have consumed block n-PBUF
                    vector.wait_ge(red_sem, n - PBUF + 1)
                nc.vector.tensor_mul(
                    prods[n % PBUF][:], ebufs[n % NBUF][:], tb[:]
                ).then_inc(mul_sem, 1)

        @block.scalar
        def _(scalar):
            # warm the ACT function table while idle (lazy-loads ~1.3 us on
            # first ACTIVATE otherwise)
            nc.scalar.activation(
                out=junk[0:1, 0:1],
                in_=junk[0:1, 0:1],
                func=mybir.ActivationFunctionType.Copy,
            )
            # right half of the target broadcast on ACT's own HWDGE ring
            scalar.dma_start(
                tb[:, H2:], dec[L - 1 : L, H2:].to_broadcast((P, H - H2))
            ).then_inc(tbr_sem, 16)
            for n in range(N_BLOCKS):
                scalar.wait_ge(mul_sem, n + 1)
                nc.scalar.activation(
                    out=junk[:],
                    in_=prods[n % PBUF][:],
                    func=mybir.ActivationFunctionType.Copy,
                    accum_out=sc[:, n : n + 1],
                ).then_inc(red_sem, 1)

    return nc


def kernel(encoder_out: np.ndarray, decoder_hidden: np.ndarray) -> np.ndarray:
    global LAST_RESULT
    from concourse.bass_utils import run_bass_kernel_spmd

    encoder_out = np.ascontiguousarray(np.asarray(encoder_out, dtype=np.float32))
    decoder_hidden = np.ascontiguousarray(np.asarray(decoder_hidden, dtype=np.float32))

    if "nc" not in _NC_CACHE:
        _NC_CACHE["nc"] = _build_nc()
    nc = _NC_CACHE["nc"]

    in_maps = [
        {"enc": encoder_out[c * S_LOC : (c + 1) * S_LOC], "dec": decoder_hidden}
        for c in range(N_CORES)
    ]
    res = run_bass_kernel_spmd(nc, in_maps, core_ids=list(range(N_CORES)))
    LAST_RESULT = res

    # out[p, n] = scores[n*128 + p] within each shard
    parts = [np.asarray(r["out"]).T.reshape(-1) for r in res.results]
    return np.concatenate(parts).astype(np.float32)
